# revision 1
# baseline (speedup 1.0000x reference)
"""Multi-head causal attention (d_model=768, H=12, T=4096) on 8 TRN2 NeuronCores.

Sharding (SPMD, one program on all 8 cores, data-driven differences only):
  - core c owns full head A = c (heads 0-7): all 8 query tiles of 512 rows.
  - core c owns half head B = 8 + c//2, parity p = c%2: query tiles g = 2i+p
    (i=0..3), so heads 8-11 are each split across two cores by query-tile
    parity. Causal structure is uniform across cores; parity enters only
    through host-prepared data (gathered x columns and mask tiles).
All matmuls bf16 with f32 PSUM accumulation. Scores stay transposed
(S^T [kv, q]) so no transposes are needed anywhere except V, which is
transposed on-chip with the DMA xbar. Softmax uses a fixed max (exp(s/8 - 12))
so AV accumulates across kv chunks in PSUM with no online rescaling; the
denominator comes from 64 ones-columns prepended to V (rows 0:64 of the AV
psum = l replicated). Each core writes its partial output projection
(transposed, f32); the host sums the 8 partials.
"""
import sys
sys.path.insert(0, '/opt/trn_rl_repo')
from contextlib import ExitStack

import numpy as np
import ml_dtypes

import concourse.bass as bass
import concourse.tile as tile
from concourse import bacc, mybir
from concourse.bass_utils import run_bass_kernel_spmd

BF = mybir.dt.bfloat16
F32 = mybir.dt.float32
AF = mybir.ActivationFunctionType
ts = bass.ts
BFNP = ml_dtypes.bfloat16

T = 4096          # sequence length
C = 768           # d_model
H = 12            # heads
DH = 64           # head dim
NCC = C // 128    # 6 contraction chunks of 128
TQ = 512          # query tile rows
NQT = T // TQ     # 8 query tiles (full head slot)
NBQT = 4          # query tiles in the half-head slot
KCH = 128         # kv chunk rows
EXP_BIAS = -12.0  # fixed softmax max estimate
EXP_SCALE = 0.125  # 1/sqrt(64)

_CACHE = {}


def _emit_kv_proj(nc, pj, wkv_sb, col_off, xt_sb, kt_sb, vt_sb):
    """kv^T projection over full T rows; kt replicated to both halves,
    v^T written to rows 64:128 of vt_sb."""
    for rt in range(NQT):
        ps = pj.tile([128, TQ], F32, tag="pj")
        for ci in range(NCC):
            nc.tensor.matmul(ps[:], lhsT=wkv_sb[:, ci, col_off:col_off + 128],
                             rhs=xt_sb[:, ci, ts(rt, TQ)],
                             start=(ci == 0), stop=(ci == NCC - 1))
        nc.vector.tensor_copy(kt_sb[0:64, ts(rt, TQ)], ps[0:64, :])
        nc.vector.tensor_copy(vt_sb[64:128, ts(rt, TQ)], ps[64:128, :])
        nc.sync.dma_start(out=kt_sb[64:128, ts(rt, TQ)], in_=kt_sb[0:64, ts(rt, TQ)])


def _emit_q_proj(nc, pj, wq_sb, col_off, x_sb, qt_sb, n_rt):
    for rt in range(n_rt):
        ps = pj.tile([128, TQ], F32, tag="pj")
        for ci in range(NCC):
            nc.tensor.matmul(ps[0:64, :], lhsT=wq_sb[:, ci, col_off:col_off + 64],
                             rhs=x_sb[:, ci, ts(rt, TQ)],
                             start=(ci == 0), stop=(ci == NCC - 1))
        nc.vector.tensor_copy(qt_sb[0:64, ts(rt, TQ)], ps[0:64, :])
        nc.sync.dma_start(out=qt_sb[64:128, ts(rt, TQ)], in_=qt_sb[0:64, ts(rt, TQ)])


def _emit_vprime(nc, vt_sb, vp_sb):
    """vp[:, ch, 0:64] = 1.0 (denominator columns); vp[:, ch, 64:128] = V chunk."""
    nc.vector.memset(vp_sb[:, :, 0:64], 1.0)
    for ch in range(T // KCH):
        nc.sync.dma_start(out=vp_sb[:, ch, 64:128],
                          in_=vt_sb[64:128, ts(ch, KCH)], transpose=True)


def _emit_attn_qtile(nc, sp, avp, ap_pool, ln_pool, ebias,
                     kt_sb, vp_sb, qt_sb, qt_idx, nch, mask_sb, nmask, nh_sb, nh_idx):
    """One query tile: QK^T over nch kv chunks, exp, mask, AV accumulate, normalize."""
    q0 = qt_sb[0:64, ts(qt_idx, TQ)]
    q1 = qt_sb[64:128, ts(qt_idx, TQ)]
    av = avp.tile([128, TQ], F32, tag="av")
    ngrp = nch // 2
    for g in range(ngrp):
        s_ps = sp.tile([128, 2, TQ], F32, tag="s")
        c0, c1 = 2 * g, 2 * g + 1
        nc.tensor.matmul(s_ps[:, 0, :], lhsT=kt_sb[0:64, ts(c0, KCH)], rhs=q0,
                         start=True, stop=True, skip_group_check=True)
        nc.tensor.matmul(s_ps[:, 1, :], lhsT=kt_sb[64:128, ts(c1, KCH)], rhs=q1,
                         start=True, stop=True, skip_group_check=True)
        a_sb = ap_pool.tile([128, 2, TQ], BF, tag="a")
        nc.scalar.activation(a_sb[:], s_ps[:], AF.Exp, bias=ebias[:], scale=EXP_SCALE)
        for j in (0, 1):
            mi = (2 * g + j) - (nch - nmask)
            if mi >= 0:
                nc.vector.tensor_mul(a_sb[:, j, :], a_sb[:, j, :], mask_sb[:, mi, :])
        for j in (0, 1):
            nc.tensor.matmul(av[:], lhsT=vp_sb[:, 2 * g + j, :], rhs=a_sb[:, j, :],
                             start=(g == 0 and j == 0), stop=(g == ngrp - 1 and j == 1),
                             skip_group_check=True)
    linv = ln_pool.tile([128, TQ], F32, tag="linv")
    nc.vector.reciprocal(linv[0:64, :], av[0:64, :])
    nc.sync.dma_start(out=linv[64:128, :], in_=linv[0:64, :])
    nc.vector.tensor_mul(nh_sb[64:128, nh_idx, :], av[64:128, :], linv[64:128, :])


def _emit_outproj(nc, pj, out_pool, wp_sb, nh_sb, n_qt, bias_sb, out_dram, use_bias):
    for t in range(n_qt):
        for cc in range(NCC):
            ps = pj.tile([128, TQ], F32, tag="pj")
            nc.tensor.matmul(ps[:], lhsT=wp_sb[64:128, ts(cc, 128)],
                             rhs=nh_sb[64:128, t, :], start=True, stop=True)
            o_sb = out_pool.tile([128, TQ], F32, tag="o")
            if use_bias:
                nc.vector.tensor_scalar_add(o_sb[:], ps[:], bias_sb[:, cc:cc + 1])
            else:
                nc.vector.tensor_copy(o_sb[:], ps[:])
            nc.sync.dma_start(out=out_dram[ts(cc, 128), ts(t, TQ)], in_=o_sb[:])


def _build_program():
    if "nc" in _CACHE:
        return _CACHE["nc"]
    nc = bacc.Bacc("TRN2", target_bir_lowering=False, debug=False, num_devices=8)
    io = {}
    for name, shape, dt in [
        ("xt", [C, T], BF), ("xtb", [C, T // 2], BF),
        ("wkv", [C, 4 * DH], BF), ("wq", [C, 2 * DH], BF),
        ("wpa", [DH, C], BF), ("wpb", [DH, C], BF),
        ("maska", [128, 4 * TQ], BF), ("maskb", [128, 8 * TQ], BF),
        ("biasv", [128, NCC], F32),
    ]:
        io[name] = nc.dram_tensor(name, shape, dt, kind="ExternalInput").ap()
    io["outa"] = nc.dram_tensor("outa", [C, T], F32, kind="ExternalOutput").ap()
    io["outb"] = nc.dram_tensor("outb", [C, T // 2], F32, kind="ExternalOutput").ap()

    with tile.TileContext(nc) as tc, ExitStack() as ctx:
        const = ctx.enter_context(tc.tile_pool(name="const", bufs=1))
        slab = ctx.enter_context(tc.tile_pool(name="slab", bufs=1))
        pj = ctx.enter_context(tc.tile_pool(name="pj", bufs=2, space="PSUM"))
        sp = ctx.enter_context(tc.tile_pool(name="sp", bufs=2, space="PSUM"))
        avp = ctx.enter_context(tc.tile_pool(name="avp", bufs=2, space="PSUM"))
        ap_pool = ctx.enter_context(tc.tile_pool(name="apool", bufs=3))
        ln_pool = ctx.enter_context(tc.tile_pool(name="lnpool", bufs=2))
        out_pool = ctx.enter_context(tc.tile_pool(name="outpool", bufs=3))

        # ---- constants / inputs to SBUF
        xt_sb = const.tile([128, NCC, T], BF)
        xtb_sb = const.tile([128, NCC, T // 2], BF)
        for ci in range(NCC):
            nc.sync.dma_start(out=xt_sb[:, ci, :], in_=io["xt"][ts(ci, 128), :])
            nc.sync.dma_start(out=xtb_sb[:, ci, :], in_=io["xtb"][ts(ci, 128), :])
        wkv_sb = const.tile([128, NCC, 4 * DH], BF)
        wq_sb = const.tile([128, NCC, 2 * DH], BF)
        for ci in range(NCC):
            nc.sync.dma_start(out=wkv_sb[:, ci, :], in_=io["wkv"][ts(ci, 128), :])
            nc.sync.dma_start(out=wq_sb[:, ci, :], in_=io["wq"][ts(ci, 128), :])
        wpa_sb = const.tile([128, C], BF)
        wpb_sb = const.tile([128, C], BF)
        nc.sync.dma_start(out=wpa_sb[64:128, :], in_=io["wpa"])
        nc.sync.dma_start(out=wpb_sb[64:128, :], in_=io["wpb"])
        maska_sb = const.tile([128, 4, TQ], BF)
        maskb_sb = const.tile([128, 8, TQ], BF)
        nc.sync.dma_start(out=maska_sb[:], in_=io["maska"])
        nc.sync.dma_start(out=maskb_sb[:], in_=io["maskb"])
        bias_sb = const.tile([128, NCC], F32)
        nc.sync.dma_start(out=bias_sb[:], in_=io["biasv"])
        ebias = const.tile([128, 1], F32)
        nc.vector.memset(ebias[:], EXP_BIAS)

        # ---- slot buffers
        ktA = slab.tile([128, T], BF)
        vtA = slab.tile([128, T], BF)
        vpA = slab.tile([128, T // KCH, 128], BF)
        qtA = slab.tile([128, T], BF)
        nhA = slab.tile([128, NQT, TQ], BF)
        ktB = slab.tile([128, T], BF)
        vtB = slab.tile([128, T], BF)
        vpB = slab.tile([128, T // KCH, 128], BF)
        qtB = slab.tile([128, T // 2], BF)
        nhB = slab.tile([128, NBQT, TQ], BF)

        # ---- projections
        _emit_kv_proj(nc, pj, wkv_sb, 0, xt_sb, ktA, vtA)
        _emit_q_proj(nc, pj, wq_sb, 0, xt_sb, qtA, NQT)
        _emit_vprime(nc, vtA, vpA)
        _emit_kv_proj(nc, pj, wkv_sb, 128, xt_sb, ktB, vtB)
        _emit_q_proj(nc, pj, wq_sb, 64, xtb_sb, qtB, NBQT)
        _emit_vprime(nc, vtB, vpB)

        # ---- attention + output projection, head A (full causal)
        for t in range(NQT):
            _emit_attn_qtile(nc, sp, avp, ap_pool, ln_pool, ebias,
                             ktA, vpA, qtA, t, 4 * t + 4, maska_sb, 4, nhA, t)
        _emit_outproj(nc, pj, out_pool, wpa_sb, nhA, NQT, bias_sb, io["outa"], True)

        # ---- attention + output projection, head B (parity half)
        for i in range(NBQT):
            _emit_attn_qtile(nc, sp, avp, ap_pool, ln_pool, ebias,
                             ktB, vpB, qtB, i, 8 * i + 8, maskb_sb, 8, nhB, i)
        _emit_outproj(nc, pj, out_pool, wpb_sb, nhB, NBQT, bias_sb, io["outb"], False)

    nc.compile()
    _CACHE["nc"] = nc
    return nc


def _make_masks():
    ki = np.arange(128)[:, None]
    qi = np.arange(TQ)[None, :]
    tri = [(qi >= 128 * j + ki).astype(BFNP) for j in range(4)]
    maska = np.concatenate(tri, axis=1)                     # [128, 2048]
    zeros = np.zeros((128, TQ), BFNP)
    ones = np.ones((128, TQ), BFNP)
    maskb0 = np.concatenate(tri + [zeros] * 4, axis=1)      # parity 0
    maskb1 = np.concatenate([ones] * 4 + tri, axis=1)       # parity 1
    return maska, maskb0, maskb1


def kernel(x, w_qkv, w_proj, b_proj):
    nc = _build_program()
    xm = np.asarray(x, np.float32)[0]                       # [T, C]
    xt = np.ascontiguousarray(xm.T).astype(BFNP)            # [C, T]
    wq3 = np.asarray(w_qkv, np.float32).reshape(C, H, 3, DH)
    wp = np.asarray(w_proj, np.float32)
    maska, maskb0, maskb1 = _make_masks()
    in_maps = []
    for c in range(8):
        hA, hB, p = c, 8 + c // 2, c % 2
        gs = [2 * i + p for i in range(NBQT)]
        xtb = np.concatenate([xt[:, TQ * g:TQ * (g + 1)] for g in gs], axis=1)
        wkv = np.concatenate([wq3[:, hA, 1], wq3[:, hA, 2],
                              wq3[:, hB, 1], wq3[:, hB, 2]], axis=1).astype(BFNP)
        wq = np.concatenate([wq3[:, hA, 0], wq3[:, hB, 0]], axis=1).astype(BFNP)
        wpa = wp[DH * hA:DH * (hA + 1)].astype(BFNP)
        wpb = wp[DH * hB:DH * (hB + 1)].astype(BFNP)
        biasv = (np.asarray(b_proj, np.float32).reshape(NCC, 128).T.copy()
                 if c == 0 else np.zeros((128, NCC), np.float32))
        in_maps.append({
            "xt": xt, "xtb": np.ascontiguousarray(xtb),
            "wkv": np.ascontiguousarray(wkv), "wq": np.ascontiguousarray(wq),
            "wpa": np.ascontiguousarray(wpa), "wpb": np.ascontiguousarray(wpb),
            "maska": np.ascontiguousarray(maska),
            "maskb": np.ascontiguousarray(maskb0 if p == 0 else maskb1),
            "biasv": biasv,
        })
    res = run_bass_kernel_spmd(nc, in_maps, core_ids=list(range(8)))
    outT = np.zeros((C, T), np.float64)
    for c in range(8):
        outT += res.results[c]["outa"].astype(np.float64)
        p = c % 2
        ob = res.results[c]["outb"]
        for i in range(NBQT):
            g = 2 * i + p
            outT[:, TQ * g:TQ * (g + 1)] += ob[:, TQ * i:TQ * (i + 1)].astype(np.float64)
    return np.ascontiguousarray(outT.T).astype(np.float32).reshape(1, T, C)
